# revision 8
# baseline (speedup 1.0000x reference)
"""Trainium2 Bass kernel for nn_Attention_module_17179869882.

Bahdanau-style attention over encoder outputs with length masking:
    h = hidden[-1]                                  # [B, D]
    energy = tanh(h @ Wh + enc @ We + b)            # [B, S, D]
    scores = energy @ v, masked to s < len[b]       # [B, S]
    attnw  = softmax(scores)                        # [B, S]
    ctx    = attnw @ enc                            # [B, 1, E]
    returns (ctx, attnw)

B=64, S=2048, E=D=512. Data-parallel over batch on 8 NeuronCores
(8 batches/core). Masked positions (s >= len) have exactly-zero
attention weight, so their encoder rows are never loaded or computed:
batches are assigned to per-core "slots" by descending length and each
slot's s-trip-count is baked into the (single, SPMD) compiled program.

Per-core device pipeline, per slot:
  DMA(HBM f32 -> SBUF bf16 cast) -> PE transpose(128x128 blocks) ->
  bf16 energy matmul (PSUM f32) -> ACT tanh fused with per-d bias ->
  fp32r scores matmul -> DVE tensor_mask_reduce (mask + negmax) ->
  ACT exp (+sum accum) -> PE context matmul on unnormalized weights ->
  DVE scale by 1/Z.
"""
import math
import os
from contextlib import ExitStack

import numpy as np
import ml_dtypes

B, S, E, D = 64, 2048, 512, 512
NCORES = 8
BL = B // NCORES          # batches (slots) per core
SB = 512                  # s-block for matmuls
ST = 128                  # s-tile for transposes
TMAX = S // SB            # 4

_cache = {}
last_exec_ns = None


def _build(T, stage=99):
    """Build the SPMD Bass program for slot trip counts T (tuple of 8 ints,
    T[k] = number of 512-row s-blocks processed for slot k).

    stage (debug): 1=consts+cbT, 2=+dma/transpose, 3=+energy/tanh/scores,
    4=+softmax/attnw, 99=full."""
    import concourse.bass as bass
    import concourse.tile as tile
    from concourse import bacc, mybir

    dt = mybir.dt
    f32, bf16, f32r = dt.float32, dt.bfloat16, dt.float32r

    nc = bacc.Bacc("TRN2", target_bir_lowering=False, debug=False,
                   num_devices=NCORES)

    enc_d = nc.dram_tensor("enc", [BL, 128, TMAX * 4, SB], f32,
                           kind="ExternalInput").ap()
    hT_d = nc.dram_tensor("hT", [128, 4, BL], f32, kind="ExternalInput").ap()
    wh_d = nc.dram_tensor("wh", [128, 4, D], f32, kind="ExternalInput").ap()
    we_d = nc.dram_tensor("we", [128, 4, D], bf16, kind="ExternalInput").ap()
    ab_d = nc.dram_tensor("ab", [1, D], f32, kind="ExternalInput").ap()
    v4_d = nc.dram_tensor("v4", [128, 4], f32r, kind="ExternalInput").ap()
    ones_d = nc.dram_tensor("ones8", [1, BL], f32, kind="ExternalInput").ap()
    idb_d = nc.dram_tensor("identb", [128, 128], bf16, kind="ExternalInput").ap()
    idf_d = nc.dram_tensor("identf", [1, 1], f32, kind="ExternalInput").ap()
    mask_d = nc.dram_tensor("mask", [BL, S], f32, kind="ExternalInput").ap()
    attnw_d = nc.dram_tensor("attnw", [BL, S], f32, kind="ExternalOutput").ap()
    ctx_d = nc.dram_tensor("ctx", [BL, E], f32, kind="ExternalOutput").ap()

    PS = bass.MemorySpace.PSUM

    with tile.TileContext(nc) as tc, ExitStack() as ctx:
        consts = ctx.enter_context(tc.tile_pool(name="consts", bufs=1))
        encn_p = ctx.enter_context(tc.tile_pool(name="encn", bufs=4))
        encT_p = ctx.enter_context(tc.tile_pool(name="encT", bufs=2))
        th_p = ctx.enter_context(tc.tile_pool(name="th", bufs=3))
        row_p = ctx.enter_context(tc.tile_pool(name="row", bufs=2))
        sm_p = ctx.enter_context(tc.tile_pool(name="sm", bufs=4))
        wt_p = ctx.enter_context(tc.tile_pool(name="wt", bufs=2))
        eps_p = ctx.enter_context(tc.tile_pool(name="eps", bufs=2, space=PS))
        trps_p = ctx.enter_context(tc.tile_pool(name="trps", bufs=2, space=PS))
        sps_p = ctx.enter_context(tc.tile_pool(name="sps", bufs=2, space=PS))
        cps_p = ctx.enter_context(tc.tile_pool(name="cps", bufs=2, space=PS))

        # ---- constants ----
        we_t = consts.tile([128, 4, D], bf16)
        nc.sync.dma_start(we_t[:], we_d[:])
        wh_t = consts.tile([128, 4, D], f32)
        nc.sync.dma_start(wh_t[:], wh_d[:])
        hT_t = consts.tile([128, 4, BL], f32)
        nc.sync.dma_start(hT_t[:], hT_d[:])
        ab_t = consts.tile([1, D], f32)
        nc.sync.dma_start(ab_t[:], ab_d[:])
        v4_t = consts.tile([128, 4], f32r)
        nc.sync.dma_start(v4_t[:], v4_d[:])
        ones_t = consts.tile([1, BL], f32)
        nc.sync.dma_start(ones_t[:], ones_d[:])
        idb_t = consts.tile([128, 128], bf16)
        nc.sync.dma_start(idb_t[:], idb_d[:])
        idf_t = consts.tile([1, 1], f32)
        nc.sync.dma_start(idf_t[:], idf_d[:])

        # ---- cbT[d, k] = (h_k @ Wh)[d] + attn_b[d], laid out [128, 4dc, BL]
        cbT = consts.tile([128, 4, BL], f32)
        for dc in range(4):
            cb_ps = cps_p.tile([128, BL], f32, tag="cps")
            for kc in range(4):
                nc.tensor.matmul(cb_ps[:], wh_t[:, kc, dc * 128:(dc + 1) * 128],
                                 hT_t[:, kc, :], start=(kc == 0), stop=False)
            nc.tensor.matmul(cb_ps[:], ab_t[0:1, dc * 128:(dc + 1) * 128],
                             ones_t[:], start=False, stop=True)
            nc.scalar.copy(cbT[:, dc, :], cb_ps[:])

        # ---- per-slot state emitted with a one-slot software pipeline:
        # epilogue (softmax + context) of slot k is emitted after the
        # energy/scores of slot k+1 so PE never stalls on ACT/DVE.
        state = {}

        def emit_main(k):
            T4 = 4 * T[k]
            encn = encn_p.tile([128, TMAX * 4, SB], bf16, tag="encn")
            nc.gpsimd.dma_start(encn[:, 0:T4, :], enc_d[k, :, 0:T4, :])
            if stage < 2:
                state[k] = (encn, None)
                return

            encT = encT_p.tile([128, 4, TMAX * 4, 128], bf16, tag="encT")
            for t0 in range(0, T4, 4):
                for ec in range(4):
                    trp = trps_p.tile([128, 4, 128], bf16, tag="trps")
                    for j in range(4):
                        nc.tensor.transpose(
                            trp[:, j, :],
                            encn[:, t0 + j, ec * 128:(ec + 1) * 128],
                            idb_t[:])
                    nc.vector.tensor_copy(encT[:, ec, t0:t0 + 4, :], trp[:])

            if stage < 3:
                state[k] = (encn, None)
                return
            scores = row_p.tile([1, S], f32, tag="scores")
            mrow = row_p.tile([1, S], f32, tag="mrow")
            nc.sync.dma_start(mrow[0:1, 0:T[k] * SB], mask_d[k:k + 1, 0:T[k] * SB])
            for sb in range(T[k]):
                sps = sps_p.tile([1, SB], f32, tag="sps")
                for dc in range(4):
                    eps = eps_p.tile([128, SB], f32, tag="eps")
                    for ec in range(4):
                        nc.tensor.matmul(
                            eps[:],
                            we_t[:, ec, dc * 128:(dc + 1) * 128],
                            encT[:, ec, 4 * sb:4 * sb + 4, :],
                            start=(ec == 0), stop=(ec == 3))
                    th = th_p.tile([128, SB], f32r, tag="th")
                    nc.scalar.activation(th[:], eps[:],
                                         mybir.ActivationFunctionType.Tanh,
                                         bias=cbT[:, dc, k:k + 1], scale=1.0)
                    nc.tensor.matmul(sps[:], v4_t[:, dc:dc + 1], th[:],
                                     start=(dc == 0), stop=(dc == 3))
                nc.vector.tensor_add(scores[0:1, sb * SB:(sb + 1) * SB], sps[:],
                                     mrow[0:1, sb * SB:(sb + 1) * SB])
            state[k] = (encn, scores)

        def emit_epilogue(k):
            encn, scores = state.pop(k)
            if stage < 4 or scores is None:
                nc.vector.tensor_copy(
                    row_p.tile([1, S], f32, tag="msc")[0:1, 0:SB],
                    encn[0:1, 0, :].bitcast(f32).__getitem__((slice(0,1), slice(0,256)))
                ) if False else None
                return
            T4 = 4 * T[k]
            W = T[k] * SB
            negmax = sm_p.tile([1, 1], f32, tag="negmax")
            nc.vector.tensor_reduce(negmax[:], scores[0:1, 0:W],
                                    axis=mybir.AxisListType.X,
                                    op=mybir.AluOpType.max, negate=True)
            if stage == 4:
                return
            expw = row_p.tile([1, S], f32, tag="expw")
            sumexp = sm_p.tile([1, 1], f32, tag="sumexp")
            nc.scalar.activation(expw[0:1, 0:W], scores[0:1, 0:W],
                                 mybir.ActivationFunctionType.Exp,
                                 bias=negmax[:], scale=1.0,
                                 accum_out=sumexp[:])
            if stage == 5:
                return
            rec = sm_p.tile([1, 1], f32, tag="rec")
            nc.vector.reciprocal(rec[:], sumexp[:])

            # attention weights out: expw * (1/Z); tail cols are exactly 0
            # and are filled host-side.
            awr = row_p.tile([1, S], f32, tag="awr")
            nc.vector.tensor_scalar_mul(awr[0:1, 0:W], expw[0:1, 0:W], rec[:])
            nc.sync.dma_start(attnw_d[k:k + 1, 0:W], awr[0:1, 0:W])
            if stage == 6:
                return

            if stage < 5:
                return
            # w tiles: transpose expw rows into [128, T4] and cast to bf16
            wps = trps_p.tile([128, TMAX * 4], f32, tag="trps")
            for t in range(T4):
                nc.tensor.transpose(wps[:, t:t + 1],
                                    expw[0:1, t * 128:(t + 1) * 128],
                                    idf_t[:])
            wbf = wt_p.tile([128, TMAX * 4], bf16, tag="wbf")
            nc.vector.tensor_copy(wbf[:, 0:T4], wps[:, 0:T4])

            cps = cps_p.tile([1, E], f32, tag="cps")
            for st in range(T4):
                nc.tensor.matmul(cps[:], wbf[:, st:st + 1], encn[:, st, :],
                                 start=(st == 0), stop=(st == T4 - 1))
            ctxr = sm_p.tile([1, E], f32, tag="ctxr")
            nc.vector.tensor_scalar_mul(ctxr[:], cps[:], rec[:])
            nc.sync.dma_start(ctx_d[k:k + 1, :], ctxr[:])

        for k in range(BL):
            emit_main(k)
            if k >= 1:
                emit_epilogue(k - 1)
        emit_epilogue(BL - 1)

    nc.compile()
    return nc


def kernel(hidden, encoder_outputs, src_lengths, attn_w, attn_b, v):
    global last_exec_ns
    from concourse.bass_utils import run_bass_kernel_spmd

    hidden = np.asarray(hidden, dtype=np.float32)
    enc = np.ascontiguousarray(np.asarray(encoder_outputs, dtype=np.float32))
    lens = np.asarray(src_lengths).astype(np.int64)
    attn_w = np.asarray(attn_w, dtype=np.float32)
    attn_b = np.asarray(attn_b, dtype=np.float32)
    v = np.asarray(v, dtype=np.float32)

    # ---- slot assignment: sort batches by length desc, deal slot-major.
    order = np.argsort(-lens, kind="stable")
    Tb = np.maximum(1, np.ceil(lens / SB).astype(int))
    T = tuple(int(Tb[order[k * NCORES]]) for k in range(BL))

    key = T
    if key not in _cache:
        _cache[key] = _build(T)
    nc = _cache[key]

    h = hidden[-1]                       # [B, D]
    Wh, We = attn_w[:D], attn_w[D:]

    we_host = np.ascontiguousarray(
        We.reshape(4, 128, D).transpose(1, 0, 2)).astype(ml_dtypes.bfloat16)
    wh_host = np.ascontiguousarray(Wh.reshape(4, 128, D).transpose(1, 0, 2))
    ab_host = attn_b.reshape(1, D)
    v4_host = np.ascontiguousarray(v.reshape(4, 128).T)        # [128, 4]
    ones_host = np.ones((1, BL), dtype=np.float32)
    idb_host = np.eye(128, dtype=np.float32).astype(ml_dtypes.bfloat16)
    idf_host = np.ones((1, 1), dtype=np.float32)

    in_maps = []
    batch_ids = []
    for c in range(NCORES):
        bks = [int(order[k * NCORES + c]) for k in range(BL)]
        batch_ids.append(bks)
        enc_pack = np.empty((BL, 128, TMAX * 4, SB), dtype=np.float32)
        hT_host = np.empty((128, 4, BL), dtype=np.float32)
        mask_host = np.zeros((BL, S), dtype=np.float32)
        for k, bk in enumerate(bks):
            T4 = 4 * T[k]
            rows = enc[bk, :T4 * ST].reshape(T4, ST, E).transpose(1, 0, 2)
            enc_pack[k, :, :T4, :] = rows
            hT_host[:, :, k] = h[bk].reshape(4, 128).T
            mask_host[k, lens[bk]:] = -1.0e10
        in_maps.append({
            "enc": enc_pack,
            "hT": hT_host,
            "wh": wh_host,
            "we": we_host,
            "ab": ab_host,
            "v4": v4_host,
            "ones8": ones_host,
            "identb": idb_host,
            "identf": idf_host,
            "mask": mask_host,
        })

    trace = bool(os.environ.get("KERNEL_TRACE"))
    if trace:
        try:
            import types, sys
            if "antenv.axon_hooks" not in sys.modules:
                from trn_agent_boot.trn_boot import _ntff_profile_via_ctypes
                mod = types.ModuleType("antenv.axon_hooks")
                hook = _ntff_profile_via_ctypes("/opt/axon/libaxon_pjrt.so")
                mod.get_axon_ntff_profile_hook = lambda: hook
                mod.set_axon_ntff_profile_hook = lambda h: None
                sys.modules["antenv.axon_hooks"] = mod
                import antenv
                antenv.axon_hooks = mod
        except Exception:
            trace = False

    res = run_bass_kernel_spmd(nc, in_maps, core_ids=list(range(NCORES)),
                               trace=trace)
    last_exec_ns = res.exec_time_ns

    ctx_full = np.empty((B, 1, E), dtype=np.float32)
    attnw_full = np.zeros((B, S), dtype=np.float32)
    for c in range(NCORES):
        for k, bk in enumerate(batch_ids[c]):
            W = T[k] * SB
            ctx_full[bk, 0, :] = res.results[c]["ctx"][k]
            attnw_full[bk, :W] = res.results[c]["attnw"][k][:W]
    return ctx_full, attnw_full


# revision 23
# speedup vs baseline: 1.3201x; 1.3201x over previous
"""Trainium2 Bass kernel for nn_Attention_module_17179869882.

Bahdanau-style attention with length masking over encoder outputs:
    h = hidden[-1]                                  # [B, D]
    energy = tanh(h @ Wh + enc @ We + b)            # [B, S, D]
    scores = energy @ v, masked to s < len[b]       # [B, S]
    attnw  = softmax(scores)                        # [B, S]
    ctx    = attnw @ enc                            # [B, 1, E]
    returns (ctx, attnw)

B=64, S=2048, E=D=512. Data-parallel over batch on 8 NeuronCores
(8 batch-slots per core). Masked positions (s >= len) have exactly-zero
attention weight, so their encoder rows are never loaded or computed:
batches are assigned to per-core slots by descending length and each
slot's s-tile trip count (128-row granularity) is baked into the single
SPMD program; the host deals batches so all cores run identical trip
counts.

Per-core pipeline, per slot: chained DMA (HBM f32 -> SBUF bf16 cast) ->
PE transpose (128x128 blocks) -> bf16 energy matmul (PSUM f32) -> ACT
tanh fused with the per-d bias (h@Wh+b, computed on chip) -> fp32r
scores matmul -> additive mask folded into the PSUM->SBUF scores copy ->
max-reduce / exp(+sum accumulator) -> PE context matmul on unnormalized
exp weights -> scale by 1/Z. Epilogues lag two slots behind so PE never
waits on ACT/DVE; a short PE warm-up bridges the initial DMA latency to
keep the HAM clock-gate at 8/8.
"""
import os
from contextlib import ExitStack

import numpy as np
import ml_dtypes

B, S, E, D = 64, 2048, 512, 512
NCORES = 8
BL = B // NCORES          # batch slots per core
ST = 128                  # s-tile (transpose granularity)
SB = 512                  # s-block (matmul free dim)
T4MAX = S // ST           # 16

_cache = {}
last_exec_ns = None
last_res = None


def _build(T4, stage=99):
    """Build the SPMD program. T4: tuple of 8 ints, slot k processes
    T4[k] 128-row s-tiles (1..16)."""
    import concourse.bass as bass
    import concourse.tile as tile
    from concourse.tile import add_dep_helper
    from concourse import bacc, mybir

    dt = mybir.dt
    f32, bf16, f32r = dt.float32, dt.bfloat16, dt.float32r

    nc = bacc.Bacc("TRN2", target_bir_lowering=False, debug=False,
                   num_devices=NCORES)

    enc_d = nc.dram_tensor("enc", [BL, 128, T4MAX, SB], f32,
                           kind="ExternalInput").ap()
    hT_d = nc.dram_tensor("hT", [128, 4, BL], bf16, kind="ExternalInput").ap()
    wh_d = nc.dram_tensor("wh", [128, 4, D], bf16, kind="ExternalInput").ap()
    we_d = nc.dram_tensor("we", [128, 4, D], bf16, kind="ExternalInput").ap()
    ab_d = nc.dram_tensor("ab", [1, D], f32, kind="ExternalInput").ap()
    v4_d = nc.dram_tensor("v4", [128, 4], f32r, kind="ExternalInput").ap()
    ones_d = nc.dram_tensor("ones8", [1, BL], f32, kind="ExternalInput").ap()
    idb_d = nc.dram_tensor("identb", [128, 128], bf16, kind="ExternalInput").ap()
    idf_d = nc.dram_tensor("identf", [1, 1], f32, kind="ExternalInput").ap()
    mask_d = nc.dram_tensor("mask", [BL, S], f32, kind="ExternalInput").ap()
    attnw_d = nc.dram_tensor("attnw", [BL, S], f32, kind="ExternalOutput").ap()
    ctx_d = nc.dram_tensor("ctx", [BL, E], f32, kind="ExternalOutput").ap()

    PS = bass.MemorySpace.PSUM

    with tile.TileContext(nc) as tc, ExitStack() as ctx:
        consts = ctx.enter_context(tc.tile_pool(name="consts", bufs=1))
        encn_p = ctx.enter_context(tc.tile_pool(name="encn", bufs=14))
        encT_p = ctx.enter_context(tc.tile_pool(name="encT", bufs=2))
        th_p = ctx.enter_context(tc.tile_pool(name="th", bufs=3))
        row_p = ctx.enter_context(tc.tile_pool(name="row", bufs=2))
        sm_p = ctx.enter_context(tc.tile_pool(name="sm", bufs=4))
        wt_p = ctx.enter_context(tc.tile_pool(name="wt", bufs=2))
        eps_p = ctx.enter_context(tc.tile_pool(name="eps", bufs=2, space=PS))
        trps_p = ctx.enter_context(tc.tile_pool(name="trps", bufs=2, space=PS))
        sps_p = ctx.enter_context(tc.tile_pool(name="sps", bufs=2, space=PS))
        cps_p = ctx.enter_context(tc.tile_pool(name="cps", bufs=2, space=PS))

        # ---- constants (identity first: the PE warm-up needs it asap) ----
        idb_t = consts.tile([128, 128], bf16)
        nc.sync.dma_start(idb_t[:], idb_d[:])
        we_t = consts.tile([128, 4, D], bf16)
        nc.sync.dma_start(we_t[:], we_d[:])
        v4_t = consts.tile([128, 4], f32r)
        nc.sync.dma_start(v4_t[:], v4_d[:])
        wh_t = consts.tile([128, 4, D], bf16)
        nc.sync.dma_start(wh_t[:], wh_d[:])
        hT_t = consts.tile([128, 4, BL], bf16)
        nc.sync.dma_start(hT_t[:], hT_d[:])
        ab_t = consts.tile([1, D], f32)
        nc.sync.dma_start(ab_t[:], ab_d[:])
        ones_t = consts.tile([1, BL], f32)
        nc.sync.dma_start(ones_t[:], ones_d[:])
        idf_t = consts.tile([1, 1], f32)
        nc.sync.dma_start(idf_t[:], idf_d[:])

        # ---- PE warm-up: keep TensorE busy through the initial enc DMA so
        # the HAM clock-gate reaches 8/8 before real work starts.
        wtile = consts.tile([128, SB], bf16)
        nc.vector.memset(wtile[:], 0)
        wps0 = cps_p.tile([128, SB], f32, tag="cps", name="warmps")
        NWARM = 16
        for i in range(NWARM):
            nc.tensor.matmul(wps0[:], idb_t[:], wtile[:],
                             start=(i == 0), stop=(i == NWARM - 1))

        # ---- cbT[d, k] = (h_k @ Wh)[d] + attn_b[d], layout [128, 4dc, BL]
        cbT = consts.tile([128, 4, BL], f32)

        def emit_cbt():
            for dc in range(4):
                cb_ps = cps_p.tile([128, BL], f32, tag="cps", name=f"cbps{dc}")
                for kc in range(4):
                    nc.tensor.matmul(cb_ps[:],
                                     wh_t[:, kc, dc * 128:(dc + 1) * 128],
                                     hT_t[:, kc, :], start=(kc == 0), stop=False)
                nc.tensor.matmul(cb_ps[:], ab_t[0:1, dc * 128:(dc + 1) * 128],
                                 ones_t[:], start=False, stop=True)
                nc.scalar.copy(cbT[:, dc, :], cb_ps[:])

        enc_dmas = []
        state = {}

        def emit_main(k):
            t4 = T4[k]
            nb = (t4 + 3) // 4                     # number of s-blocks
            rs = [min(4, t4 - 4 * sb) for sb in range(nb)]
            encn = [encn_p.tile([128, 4, SB], bf16, tag="encn",
                                name=f"encn_{k}_{sb}")
                    for sb in range(nb)]
            for sb in range(nb):
                r = rs[sb]
                di = nc.gpsimd.dma_start(encn[sb][:, 0:r, :],
                                         enc_d[k, :, 4 * sb:4 * sb + r, :])
                enc_dmas.append(di)
                if len(enc_dmas) >= 3:
                    add_dep_helper(di.ins, enc_dmas[-3].ins,
                                   reason="enc dma in-order completion")
            if stage < 2:
                state[k] = (encn, None)
                return

            encT = encT_p.tile([128, 4, T4MAX, 128], bf16, tag="encT")
            for sb in range(nb):
                r = rs[sb]
                for ec in range(4):
                    trp = trps_p.tile([128, 4, 128], bf16, tag="trps")
                    for j in range(r):
                        nc.tensor.transpose(
                            trp[:, j, :],
                            encn[sb][:, j, ec * 128:(ec + 1) * 128],
                            idb_t[:])
                    nc.vector.tensor_copy(
                        encT[:, ec, 4 * sb:4 * sb + r, :], trp[:, 0:r, :])

            if stage < 3:
                state[k] = (encn, None)
                return
            scores = row_p.tile([1, S], f32, tag="scores")
            mrow = row_p.tile([1, S], f32, tag="mrow")
            nc.sync.dma_start(mrow[0:1, 0:t4 * ST], mask_d[k:k + 1, 0:t4 * ST])
            for sb in range(nb):
                r = rs[sb]
                w = r * ST
                sps = sps_p.tile([1, SB], f32, tag="sps")
                for dc in range(4):
                    eps = eps_p.tile([128, SB], f32, tag="eps")
                    for ec in range(4):
                        nc.tensor.matmul(
                            eps[:, 0:w],
                            we_t[:, ec, dc * 128:(dc + 1) * 128],
                            encT[:, ec, 4 * sb:4 * sb + r, :],
                            start=(ec == 0), stop=(ec == 3))
                    th = th_p.tile([128, SB], f32r, tag="th")
                    nc.scalar.activation(th[:, 0:w], eps[:, 0:w],
                                         mybir.ActivationFunctionType.Tanh,
                                         bias=cbT[:, dc, k:k + 1], scale=1.0)
                    nc.tensor.matmul(sps[0:1, 0:w], v4_t[:, dc:dc + 1],
                                     th[:, 0:w],
                                     start=(dc == 0), stop=(dc == 3))
                nc.vector.tensor_add(
                    scores[0:1, sb * SB:sb * SB + w], sps[0:1, 0:w],
                    mrow[0:1, sb * SB:sb * SB + w])
            state[k] = (encn, scores)

        def emit_epilogue(k):
            encn, scores = state.pop(k)
            if stage < 4 or scores is None:
                return
            t4 = T4[k]
            W = t4 * ST
            negmax = sm_p.tile([1, 1], f32, tag="negmax")
            nc.vector.tensor_reduce(negmax[:], scores[0:1, 0:W],
                                    axis=mybir.AxisListType.X,
                                    op=mybir.AluOpType.max, negate=True)
            if stage == 4:
                return
            expw = row_p.tile([1, S], f32, tag="expw")
            sumexp = sm_p.tile([1, 1], f32, tag="sumexp")
            nc.scalar.activation(expw[0:1, 0:W], scores[0:1, 0:W],
                                 mybir.ActivationFunctionType.Exp,
                                 bias=negmax[:], scale=1.0,
                                 accum_out=sumexp[:])
            if stage == 5:
                return
            rec = sm_p.tile([1, 1], f32, tag="rec")
            nc.vector.reciprocal(rec[:], sumexp[:])

            # attention weights out: expw * (1/Z); tail cols (s >= W) are
            # exactly zero and are filled host-side.
            awr = row_p.tile([1, S], f32, tag="scores", name=f"awr{k}")
            nc.vector.tensor_scalar_mul(awr[0:1, 0:W], expw[0:1, 0:W], rec[:])
            nc.sync.dma_start(attnw_d[k:k + 1, 0:W], awr[0:1, 0:W])
            if stage == 6:
                return

            # w tiles: transpose expw rows into [128, t4], cast to bf16
            wps = trps_p.tile([128, T4MAX], f32, tag="trps")
            for t in range(t4):
                nc.tensor.transpose(wps[:, t:t + 1],
                                    expw[0:1, t * 128:(t + 1) * 128],
                                    idf_t[:])
            wbf = wt_p.tile([128, T4MAX], bf16, tag="wbf")
            nc.vector.tensor_copy(wbf[:, 0:t4], wps[:, 0:t4])

            cps = cps_p.tile([1, E], f32, tag="cps")
            for st in range(t4):
                nc.tensor.matmul(cps[:], wbf[:, st:st + 1],
                                 encn[st // 4][:, st % 4, :],
                                 start=(st == 0), stop=(st == t4 - 1))
            ctxr = sm_p.tile([1, E], f32, tag="ctxr")
            nc.vector.tensor_scalar_mul(ctxr[:], cps[:], rec[:])
            nc.sync.dma_start(ctx_d[k:k + 1, :], ctxr[:])

        ks = sorted(range(BL), key=lambda kk: T4[kk])
        ks = [ks[0]] + ks[2:] + [ks[1]]          # small first AND small last
        emit_cbt()
        LAG = 2
        for i, k in enumerate(ks):
            emit_main(k)
            if i >= LAG:
                emit_epilogue(ks[i - LAG])
        for i in range(max(0, len(ks) - LAG), len(ks)):
            emit_epilogue(ks[i])

    nc.compile()
    return nc


def kernel(hidden, encoder_outputs, src_lengths, attn_w, attn_b, v):
    global last_exec_ns, last_res
    from concourse.bass_utils import run_bass_kernel_spmd

    hidden = np.asarray(hidden, dtype=np.float32)
    enc = np.ascontiguousarray(np.asarray(encoder_outputs, dtype=np.float32))
    lens = np.asarray(src_lengths).astype(np.int64)
    attn_w = np.asarray(attn_w, dtype=np.float32)
    attn_b = np.asarray(attn_b, dtype=np.float32)
    v = np.asarray(v, dtype=np.float32)

    # ---- slot assignment: sort batches by length desc, deal slot-major.
    order = np.argsort(-lens, kind="stable")
    Tb4 = np.maximum(1, np.ceil(lens / ST).astype(int))
    T4 = tuple(int(Tb4[order[k * NCORES]]) for k in range(BL))

    key = T4
    if key not in _cache:
        _cache[key] = _build(T4)
    nc = _cache[key]

    h = hidden[-1]                       # [B, D]
    Wh, We = attn_w[:D], attn_w[D:]

    we_host = np.ascontiguousarray(
        We.reshape(4, 128, D).transpose(1, 0, 2)).astype(ml_dtypes.bfloat16)
    wh_host = np.ascontiguousarray(
        Wh.reshape(4, 128, D).transpose(1, 0, 2)).astype(ml_dtypes.bfloat16)
    ab_host = attn_b.reshape(1, D)
    v4_host = np.ascontiguousarray(v.reshape(4, 128).T)        # [128, 4]
    ones_host = np.ones((1, BL), dtype=np.float32)
    idb_host = np.eye(128, dtype=np.float32).astype(ml_dtypes.bfloat16)
    idf_host = np.ones((1, 1), dtype=np.float32)

    in_maps = []
    batch_ids = []
    for c in range(NCORES):
        bks = [int(order[k * NCORES + c]) for k in range(BL)]
        batch_ids.append(bks)
        enc_pack = np.empty((BL, 128, T4MAX, SB), dtype=np.float32)
        hT_host = np.empty((128, 4, BL), dtype=np.float32)
        mask_host = np.zeros((BL, S), dtype=np.float32)
        for k, bk in enumerate(bks):
            t4 = T4[k]
            rows = enc[bk, :t4 * ST]
            enc_pack[k, :, :t4, :] = rows.reshape(t4, ST, E).transpose(1, 0, 2)
            hT_host[:, :, k] = h[bk].reshape(4, 128).T
            mask_host[k, lens[bk]:] = -1.0e10
        in_maps.append({
            "enc": enc_pack,
            "hT": hT_host.astype(ml_dtypes.bfloat16),
            "wh": wh_host,
            "we": we_host,
            "ab": ab_host,
            "v4": v4_host,
            "ones8": ones_host,
            "identb": idb_host,
            "identf": idf_host,
            "mask": mask_host,
        })

    trace = os.environ.get("KERNEL_TRACE", "") in ("1", "true")
    if trace:
        try:
            import types, sys
            if "antenv.axon_hooks" not in sys.modules:
                from trn_agent_boot.trn_boot import _ntff_profile_via_ctypes
                mod = types.ModuleType("antenv.axon_hooks")
                hook = _ntff_profile_via_ctypes("/opt/axon/libaxon_pjrt.so")
                mod.get_axon_ntff_profile_hook = lambda: hook
                mod.set_axon_ntff_profile_hook = lambda h: None
                sys.modules["antenv.axon_hooks"] = mod
                import antenv
                antenv.axon_hooks = mod
        except Exception:
            trace = False

    res = run_bass_kernel_spmd(nc, in_maps, core_ids=list(range(NCORES)),
                               trace=trace)
    last_exec_ns = res.exec_time_ns
    last_res = res

    ctx_full = np.empty((B, 1, E), dtype=np.float32)
    attnw_full = np.zeros((B, S), dtype=np.float32)
    for c in range(NCORES):
        for k, bk in enumerate(batch_ids[c]):
            W = T4[k] * ST
            ctx_full[bk, 0, :] = res.results[c]["ctx"][k]
            attnw_full[bk, :W] = res.results[c]["attnw"][k][:W]
    return ctx_full, attnw_full


# revision 27
# speedup vs baseline: 1.4085x; 1.0670x over previous
"""Trainium2 Bass kernel for nn_Attention_module_17179869882.

Bahdanau-style attention with length masking over encoder outputs:
    h = hidden[-1]                                  # [B, D]
    energy = tanh(h @ Wh + enc @ We + b)            # [B, S, D]
    scores = energy @ v, masked to s < len[b]       # [B, S]
    attnw  = softmax(scores)                        # [B, S]
    ctx    = attnw @ enc                            # [B, 1, E]
    returns (ctx, attnw)

B=64, S=2048, E=D=512. Data-parallel over batch on 8 NeuronCores
(8 batch-slots per core). Masked positions (s >= len) have exactly-zero
attention weight, so their encoder rows are never loaded or computed:
batches are assigned to per-core slots by descending length and each
slot's s-tile trip count (128-row granularity) is baked into the single
SPMD program; the host deals batches so all cores run identical trip
counts.

Per-core pipeline, per slot: chained DMA (HBM f32 -> SBUF bf16 cast) ->
PE transpose (128x128 blocks) -> bf16 energy matmul (PSUM f32) -> ACT
tanh fused with the per-d bias (h@Wh+b, computed on chip) -> fp32r
scores matmul -> additive mask folded into the PSUM->SBUF scores copy ->
max-reduce / exp(+sum accumulator) -> PE context matmul on unnormalized
exp weights -> scale by 1/Z. Epilogues lag two slots behind so PE never
waits on ACT/DVE; a short PE warm-up bridges the initial DMA latency to
keep the HAM clock-gate at 8/8.
"""
import os
from contextlib import ExitStack

import numpy as np
import ml_dtypes

B, S, E, D = 64, 2048, 512, 512
NCORES = 8
BL = B // NCORES          # batch slots per core
ST = 128                  # s-tile (transpose granularity)
SB = 512                  # s-block (matmul free dim)
T4MAX = S // ST           # 16

_cache = {}
last_exec_ns = None
last_res = None


def _build(T4, MB, stage=99):
    """Build the SPMD program. T4: tuple of 8 ints, slot k processes
    T4[k] 128-row s-tiles (1..16). MB[k]: first s-block index that can
    contain masked positions for any batch in slot k (blocks before it
    are all-valid for every core and skip the mask add)."""
    import concourse.bass as bass
    import concourse.tile as tile
    from concourse.tile import add_dep_helper
    from concourse import bacc, mybir

    dt = mybir.dt
    f32, bf16, f32r = dt.float32, dt.bfloat16, dt.float32r

    nc = bacc.Bacc("TRN2", target_bir_lowering=False, debug=False,
                   num_devices=NCORES)

    enc_d = nc.dram_tensor("enc", [BL, 128, T4MAX, SB], f32,
                           kind="ExternalInput").ap()
    hT_d = nc.dram_tensor("hT", [128, 4, BL], bf16, kind="ExternalInput").ap()
    wh_d = nc.dram_tensor("wh", [128, 4, D], bf16, kind="ExternalInput").ap()
    we_d = nc.dram_tensor("we", [128, 4, D], bf16, kind="ExternalInput").ap()
    ab_d = nc.dram_tensor("ab", [1, D], f32, kind="ExternalInput").ap()
    v4_d = nc.dram_tensor("v4", [128, 4], f32r, kind="ExternalInput").ap()
    ones_d = nc.dram_tensor("ones8", [1, BL], f32, kind="ExternalInput").ap()
    idb_d = nc.dram_tensor("identb", [128, 128], bf16, kind="ExternalInput").ap()
    idf_d = nc.dram_tensor("identf", [1, 1], f32, kind="ExternalInput").ap()
    mask_d = nc.dram_tensor("mask", [BL, S], f32, kind="ExternalInput").ap()
    attnw_d = nc.dram_tensor("attnw", [BL, S], f32, kind="ExternalOutput").ap()
    ctx_d = nc.dram_tensor("ctx", [BL, E], f32, kind="ExternalOutput").ap()

    PS = bass.MemorySpace.PSUM

    with tile.TileContext(nc) as tc, ExitStack() as ctx:
        consts = ctx.enter_context(tc.tile_pool(name="consts", bufs=1))
        encn_p = ctx.enter_context(tc.tile_pool(name="encn", bufs=16))
        encT_p = ctx.enter_context(tc.tile_pool(name="encT", bufs=2))
        th_p = ctx.enter_context(tc.tile_pool(name="th", bufs=6))
        row_p = ctx.enter_context(tc.tile_pool(name="row", bufs=2))
        sm_p = ctx.enter_context(tc.tile_pool(name="sm", bufs=4))
        wt_p = ctx.enter_context(tc.tile_pool(name="wt", bufs=2))
        eps_p = ctx.enter_context(tc.tile_pool(name="eps", bufs=3, space=PS))
        trps_p = ctx.enter_context(tc.tile_pool(name="trps", bufs=2, space=PS))
        sps_p = ctx.enter_context(tc.tile_pool(name="sps", bufs=1, space=PS))
        cps_p = ctx.enter_context(tc.tile_pool(name="cps", bufs=2, space=PS))

        # ---- constants (identity first: the PE warm-up needs it asap) ----
        idb_t = consts.tile([128, 128], bf16)
        nc.sync.dma_start(idb_t[:], idb_d[:])
        we_t = consts.tile([128, 4, D], bf16)
        nc.sync.dma_start(we_t[:], we_d[:])
        v4_t = consts.tile([128, 4], f32r)
        nc.sync.dma_start(v4_t[:], v4_d[:])
        wh_t = consts.tile([128, 4, D], bf16)
        nc.sync.dma_start(wh_t[:], wh_d[:])
        hT_t = consts.tile([128, 4, BL], bf16)
        nc.sync.dma_start(hT_t[:], hT_d[:])
        ab_t = consts.tile([1, D], f32)
        nc.sync.dma_start(ab_t[:], ab_d[:])
        ones_t = consts.tile([1, BL], f32)
        nc.sync.dma_start(ones_t[:], ones_d[:])
        idf_t = consts.tile([1, 1], f32)
        nc.sync.dma_start(idf_t[:], idf_d[:])

        # ---- PE warm-up: keep TensorE busy through the initial enc DMA so
        # the HAM clock-gate reaches 8/8 before real work starts.
        wtile = consts.tile([128, SB], bf16)
        nc.vector.memset(wtile[:], 0)
        wps0 = cps_p.tile([128, SB], f32, tag="cps", name="warmps")
        NWARM = 30
        for i in range(NWARM):
            nc.tensor.matmul(wps0[:], idb_t[:], wtile[:],
                             start=(i == 0), stop=(i == NWARM - 1))

        # ---- cbT[d, k] = (h_k @ Wh)[d] + attn_b[d], layout [128, 4dc, BL]
        cbT = consts.tile([128, 4, BL], f32)

        def emit_cbt():
            for dc in range(4):
                cb_ps = cps_p.tile([128, BL], f32, tag="cps", name=f"cbps{dc}")
                for kc in range(4):
                    nc.tensor.matmul(cb_ps[:],
                                     wh_t[:, kc, dc * 128:(dc + 1) * 128],
                                     hT_t[:, kc, :], start=(kc == 0), stop=False)
                nc.tensor.matmul(cb_ps[:], ab_t[0:1, dc * 128:(dc + 1) * 128],
                                 ones_t[:], start=False, stop=True)
                nc.scalar.copy(cbT[:, dc, :], cb_ps[:])

        enc_dmas = []
        state = {}

        def emit_main(k):
            t4 = T4[k]
            nb = (t4 + 3) // 4                     # number of s-blocks
            rs = [min(4, t4 - 4 * sb) for sb in range(nb)]
            encn = [encn_p.tile([128, 4, SB], bf16, tag="encn",
                                name=f"encn_{k}_{sb}")
                    for sb in range(nb)]
            for sb in range(nb):
                r = rs[sb]
                di = nc.gpsimd.dma_start(encn[sb][:, 0:r, :],
                                         enc_d[k, :, 4 * sb:4 * sb + r, :])
                enc_dmas.append(di)
                if len(enc_dmas) >= 3:
                    add_dep_helper(di.ins, enc_dmas[-3].ins,
                                   reason="enc dma in-order completion")
            if stage < 2:
                state[k] = (encn, None)
                return

            encT = encT_p.tile([128, 4, T4MAX, 128], bf16, tag="encT")
            for sb in range(nb):
                r = rs[sb]
                for ec in range(4):
                    trp = trps_p.tile([128, 4, 128], bf16, tag="trps")
                    for j in range(r):
                        nc.tensor.transpose(
                            trp[:, j, :],
                            encn[sb][:, j, ec * 128:(ec + 1) * 128],
                            idb_t[:])
                    nc.vector.tensor_copy(
                        encT[:, ec, 4 * sb:4 * sb + r, :], trp[:, 0:r, :])

            if stage < 3:
                state[k] = (encn, None)
                return
            scores = row_p.tile([1, S], f32, tag="scores")
            # blocks < MB[k] are fully valid for every core; only blocks
            # >= MB[k] can contain masked positions and take the mask add.
            mb = min(MB[k], nb - 1)
            mrow = row_p.tile([1, S], f32, tag="mrow")
            mw = t4 * ST - mb * SB
            nc.sync.dma_start(mrow[0:1, 0:mw],
                              mask_d[k:k + 1, mb * SB:t4 * ST])
            for sb in range(nb):
                r = rs[sb]
                w = r * ST
                sps = sps_p.tile([1, SB], f32, tag="sps")
                ths = []
                for dc in range(4):
                    eps = eps_p.tile([128, SB], f32, tag="eps")
                    for ec in range(4):
                        nc.tensor.matmul(
                            eps[:, 0:w],
                            we_t[:, ec, dc * 128:(dc + 1) * 128],
                            encT[:, ec, 4 * sb:4 * sb + r, :],
                            start=(ec == 0), stop=(ec == 3))
                    th = th_p.tile([128, SB], f32r, tag="th")
                    nc.scalar.activation(th[:, 0:w], eps[:, 0:w],
                                         mybir.ActivationFunctionType.Tanh,
                                         bias=cbT[:, dc, k:k + 1], scale=1.0)
                    ths.append(th)
                for dc in range(4):
                    nc.tensor.matmul(sps[0:1, 0:w], v4_t[:, dc:dc + 1],
                                     ths[dc][:, 0:w],
                                     start=(dc == 0), stop=(dc == 3))
                if sb >= mb:
                    off = sb * SB - mb * SB
                    nc.vector.tensor_add(
                        scores[0:1, sb * SB:sb * SB + w], sps[0:1, 0:w],
                        mrow[0:1, off:off + w])
                else:
                    nc.vector.tensor_copy(
                        scores[0:1, sb * SB:sb * SB + w], sps[0:1, 0:w])
            state[k] = (encn, scores)

        def emit_epilogue(k):
            encn, scores = state.pop(k)
            if stage < 4 or scores is None:
                return
            t4 = T4[k]
            W = t4 * ST
            negmax = sm_p.tile([1, 1], f32, tag="negmax")
            nc.vector.tensor_reduce(negmax[:], scores[0:1, 0:W],
                                    axis=mybir.AxisListType.X,
                                    op=mybir.AluOpType.max, negate=True)
            if stage == 4:
                return
            expw = row_p.tile([1, S], f32, tag="expw")
            sumexp = sm_p.tile([1, 1], f32, tag="sumexp")
            nc.scalar.activation(expw[0:1, 0:W], scores[0:1, 0:W],
                                 mybir.ActivationFunctionType.Exp,
                                 bias=negmax[:], scale=1.0,
                                 accum_out=sumexp[:])
            if stage == 5:
                return
            rec = sm_p.tile([1, 1], f32, tag="rec")
            nc.vector.reciprocal(rec[:], sumexp[:])

            # attention weights out: expw * (1/Z); tail cols (s >= W) are
            # exactly zero and are filled host-side.
            awr = row_p.tile([1, S], f32, tag="scores", name=f"awr{k}")
            nc.vector.tensor_scalar_mul(awr[0:1, 0:W], expw[0:1, 0:W], rec[:])
            nc.sync.dma_start(attnw_d[k:k + 1, 0:W], awr[0:1, 0:W])
            if stage == 6:
                return

            # w tiles: transpose expw rows into [128, t4], cast to bf16
            wps = trps_p.tile([128, T4MAX], f32, tag="trps")
            for t in range(t4):
                nc.tensor.transpose(wps[:, t:t + 1],
                                    expw[0:1, t * 128:(t + 1) * 128],
                                    idf_t[:])
            wbf = wt_p.tile([128, T4MAX], bf16, tag="wbf")
            nc.vector.tensor_copy(wbf[:, 0:t4], wps[:, 0:t4])

            cps = cps_p.tile([1, E], f32, tag="cps")
            for st in range(t4):
                nc.tensor.matmul(cps[:], wbf[:, st:st + 1],
                                 encn[st // 4][:, st % 4, :],
                                 start=(st == 0), stop=(st == t4 - 1))
            ctxr = sm_p.tile([1, E], f32, tag="ctxr")
            nc.vector.tensor_scalar_mul(ctxr[:], cps[:], rec[:])
            nc.sync.dma_start(ctx_d[k:k + 1, :], ctxr[:])

        ks = sorted(range(BL), key=lambda kk: T4[kk])
        ks = [ks[0]] + ks[2:] + [ks[1]]          # small first AND small last
        emit_cbt()
        LAG = 2
        for i, k in enumerate(ks):
            emit_main(k)
            if i >= LAG:
                emit_epilogue(ks[i - LAG])
        for i in range(max(0, len(ks) - LAG), len(ks)):
            emit_epilogue(ks[i])

    nc.compile()
    return nc


def kernel(hidden, encoder_outputs, src_lengths, attn_w, attn_b, v):
    global last_exec_ns, last_res
    from concourse.bass_utils import run_bass_kernel_spmd

    hidden = np.asarray(hidden, dtype=np.float32)
    enc = np.ascontiguousarray(np.asarray(encoder_outputs, dtype=np.float32))
    lens = np.asarray(src_lengths).astype(np.int64)
    attn_w = np.asarray(attn_w, dtype=np.float32)
    attn_b = np.asarray(attn_b, dtype=np.float32)
    v = np.asarray(v, dtype=np.float32)

    # ---- slot assignment: sort batches by length desc, deal slot-major.
    order = np.argsort(-lens, kind="stable")
    Tb4 = np.maximum(1, np.ceil(lens / ST).astype(int))
    T4 = tuple(int(Tb4[order[k * NCORES]]) for k in range(BL))

    minlen = tuple(int(lens[order[k * NCORES + NCORES - 1]])
                   for k in range(BL))
    MB = tuple(minlen[k] // SB for k in range(BL))
    key = (T4, MB)
    if key not in _cache:
        _cache[key] = _build(T4, MB)
    nc = _cache[key]

    h = hidden[-1]                       # [B, D]
    Wh, We = attn_w[:D], attn_w[D:]

    we_host = np.ascontiguousarray(
        We.reshape(4, 128, D).transpose(1, 0, 2)).astype(ml_dtypes.bfloat16)
    wh_host = np.ascontiguousarray(
        Wh.reshape(4, 128, D).transpose(1, 0, 2)).astype(ml_dtypes.bfloat16)
    ab_host = attn_b.reshape(1, D)
    v4_host = np.ascontiguousarray(v.reshape(4, 128).T)        # [128, 4]
    ones_host = np.ones((1, BL), dtype=np.float32)
    idb_host = np.eye(128, dtype=np.float32).astype(ml_dtypes.bfloat16)
    idf_host = np.ones((1, 1), dtype=np.float32)

    in_maps = []
    batch_ids = []
    for c in range(NCORES):
        bks = [int(order[k * NCORES + c]) for k in range(BL)]
        batch_ids.append(bks)
        enc_pack = np.empty((BL, 128, T4MAX, SB), dtype=np.float32)
        hT_host = np.empty((128, 4, BL), dtype=np.float32)
        mask_host = np.zeros((BL, S), dtype=np.float32)
        for k, bk in enumerate(bks):
            t4 = T4[k]
            rows = enc[bk, :t4 * ST]
            enc_pack[k, :, :t4, :] = rows.reshape(t4, ST, E).transpose(1, 0, 2)
            hT_host[:, :, k] = h[bk].reshape(4, 128).T
            mask_host[k, lens[bk]:] = -1.0e10
        in_maps.append({
            "enc": enc_pack,
            "hT": hT_host.astype(ml_dtypes.bfloat16),
            "wh": wh_host,
            "we": we_host,
            "ab": ab_host,
            "v4": v4_host,
            "ones8": ones_host,
            "identb": idb_host,
            "identf": idf_host,
            "mask": mask_host,
        })

    trace = os.environ.get("KERNEL_TRACE", "") in ("1", "true")
    if trace:
        try:
            import types, sys
            if "antenv.axon_hooks" not in sys.modules:
                from trn_agent_boot.trn_boot import _ntff_profile_via_ctypes
                mod = types.ModuleType("antenv.axon_hooks")
                hook = _ntff_profile_via_ctypes("/opt/axon/libaxon_pjrt.so")
                mod.get_axon_ntff_profile_hook = lambda: hook
                mod.set_axon_ntff_profile_hook = lambda h: None
                sys.modules["antenv.axon_hooks"] = mod
                import antenv
                antenv.axon_hooks = mod
        except Exception:
            trace = False

    res = run_bass_kernel_spmd(nc, in_maps, core_ids=list(range(NCORES)),
                               trace=trace)
    last_exec_ns = res.exec_time_ns
    last_res = res

    ctx_full = np.empty((B, 1, E), dtype=np.float32)
    attnw_full = np.zeros((B, S), dtype=np.float32)
    for c in range(NCORES):
        for k, bk in enumerate(batch_ids[c]):
            W = T4[k] * ST
            ctx_full[bk, 0, :] = res.results[c]["ctx"][k]
            attnw_full[bk, :W] = res.results[c]["attnw"][k][:W]
    return ctx_full, attnw_full
